# revision 1
# baseline (speedup 1.0000x reference)
"""Capsule FC layer with dynamic routing on 8 Trainium2 NeuronCores.

Problem (reference.py):
  x: [B=256, N_in=1152, D_in=8], W: [N_in, N_out=10, D_out=16, D_in=8]
  u_hat = einsum('iodk,bik->biod', W, x)          # 189 MB if materialized
  4 routing rounds: c = softmax(b, o); s = squash(einsum('io,biod->bod', c, u_hat));
  b += einsum('biod,bod->io', u_hat, s) (first 3 rounds). Output s [256, 10, 16].

Strategy: shard N_in (i) 8 ways (144 capsules/core). u_hat is never
materialized; instead with j=(i,k) flattened (1152 per core) and od=(o,d)
flattened (160):
  s_partial[b, od]   = sum_j  xT[j, b] * (c[i(j), o] * W2[j, od])   (TensorE)
  s                  = AllReduce_8(s_partial)                       (164 KB)
  T[j, od]           = sum_b  x[b, j] * s[b, od]                    (TensorE)
  b_upd[i, o]        = sum_{k, d} W2[j, od] * T[j, od]              (DVE + PE)
softmax over o and the b update are fully core-local (b is i-sharded);
the only collective is one 164 KB AllReduce per routing round.

The per-partition-group reduction over k (8 rows) uses a single matmul with
a block-diagonal ones matrix (BlockOnes = kron(I_16, ones_8x8)), which both
group-sums and broadcasts the result back to all 8 k-rows.
"""

import numpy as np

B = 256
N_IN = 1152
D_IN = 8
N_OUT = 10
D_OUT = 16
N_CORES = 8
I_LOC = N_IN // N_CORES          # 144 capsules per core
J = I_LOC * D_IN                 # 1152 contraction length per core
OD = N_OUT * D_OUT               # 160
OD_PAD = 256                     # padded free dim (fp32r wants >=256 moving)
ROUTING_ITERS = 3
NCHUNK = J // 128                # 9 j-chunks
NH = B // 128                    # 2 batch halves

# matmul dtype mode: "f32" (exact, 4 cyc/row) or "f32r" (fast, 1 cyc/row @256)
MM_MODE = "f32"

_cache = {}


def _build(mm_mode, reps=1):
    import concourse.bass as bass
    import concourse.tile as tile
    from concourse import bacc
    import concourse.mybir as mybir

    f32 = mybir.dt.float32
    flags = mm_mode.split("-")
    use_f32r = flags[0] == "f32r"
    no_ar = "noar" in flags
    ar_tiny = "artiny" in flags
    use_ag = "ag" in flags
    use_rdma = "rdma" in flags
    crit_only = "critonly" in flags
    mdt = mybir.dt.float32r if use_f32r else f32
    FREE = OD_PAD if use_f32r else OD

    nc = bacc.Bacc("TRN2", target_bir_lowering=False, debug=False,
                   num_devices=N_CORES)
    x_d = nc.dram_tensor("x", [B, J], mdt, kind="ExternalInput")
    xt_d = nc.dram_tensor("xt", [J, B], mdt, kind="ExternalInput")
    w2_d = nc.dram_tensor("w2", [J, OD], f32, kind="ExternalInput")
    bones_d = nc.dram_tensor("bones", [128, 128], f32, kind="ExternalInput")
    out_d = nc.dram_tensor("out", [B, N_OUT, D_OUT], f32, kind="ExternalOutput")

    with tile.TileContext(nc, trace_sim=not use_rdma) as tc:
        with (
            tc.tile_pool(name="big", bufs=1) as big,
            tc.tile_pool(name="small", bufs=1) as small,
            tc.tile_pool(name="tmp", bufs=3) as tmp,
            tc.tile_pool(name="ps_s", bufs=1, space="PSUM") as ps_s,
            tc.tile_pool(name="ps_t", bufs=3, space="PSUM") as ps_t,
            tc.tile_pool(name="ps_b", bufs=1, space="PSUM") as ps_b,
            tc.tile_pool(name="dram", bufs=2, space="DRAM") as dram,
        ):
            # ---- load inputs -------------------------------------------------
            X = []
            for h in range(NH):
                xh = big.tile([128, J], mdt, tag=f"x{h}")
                nc.sync.dma_start(out=xh, in_=x_d[h * 128:(h + 1) * 128, :])
                X.append(xh)
            XT = []
            for c in range(NCHUNK):
                xtc = big.tile([128, B], mdt, tag=f"xt{c}")
                nc.sync.dma_start(out=xtc, in_=xt_d[c * 128:(c + 1) * 128, :])
                XT.append(xtc)
            W2 = []
            for c in range(NCHUNK):
                w2c = big.tile([128, OD], f32, tag=f"w2{c}")
                nc.sync.dma_start(out=w2c, in_=w2_d[c * 128:(c + 1) * 128, :])
                W2.append(w2c)

            # ---- constants / persistent state -------------------------------
            A2 = []
            for c in range(NCHUNK):
                a2c = big.tile([128, OD_PAD], mdt, tag=f"a2{c}")
                if use_f32r:
                    nc.vector.memset(a2c[:, OD:].bitcast(f32), 0.0)
                A2.append(a2c)

            bones = small.tile([128, 128], f32, tag="bones")
            nc.sync.dma_start(out=bones, in_=bones_d[:, :])

            b_rep = small.tile([128, NCHUNK, 16], f32, tag="b_rep")
            nc.vector.memset(b_rep[:], 0.0)
            c_rep = small.tile([128, NCHUNK, 16], f32, tag="c_rep")
            nc.vector.memset(c_rep[:], 0.0)
            nc.vector.memset(c_rep[:, :, :N_OUT], 1.0 / N_OUT)

            s_full = small.tile([128, NH, OD_PAD], mdt, tag="s_full")
            if use_f32r:
                nc.vector.memset(s_full[:].bitcast(f32), 0.0)
            s_in_ext = small.tile([128, NH * OD + 1], f32, tag="s_in_ext")
            nc.vector.memset(s_in_ext[:, NH * OD:], 0.0)
            s_in = s_in_ext[:, :NH * OD].rearrange("p (h od) -> p h od", od=OD)
            rbuf = {}
            rsems = None
            if use_rdma:
                WB = NH * OD + 1          # +1 dummy ordering column
                for par in range(2):
                    for k in range(3):
                        rbuf[(par, k)] = small.tile([128, WB], f32,
                                                    name=f"rbuf{par}{k}",
                                                    tag=f"rbuf{par}{k}")
                        nc.vector.memset(rbuf[(par, k)][:], 0.0)
                acc1 = small.tile([128, WB], f32, tag="acc1")
                nc.vector.memset(acc1[:, NH * OD:], 0.0)
                acc2 = small.tile([128, WB], f32, tag="acc2")
                nc.vector.memset(acc2[:, NH * OD:], 0.0)
                vscratch = small.tile([1, 1], f32, tag="vscratch")
                nc.vector.memset(vscratch[:], 0.0)
                rsems = [[nc.alloc_semaphore(f"rsem{k}_{p}") for p in range(2)]
                         for k in range(3)]
                rdma_lsem = nc.alloc_semaphore("rdma_lsem")
                ar_count = [0]
                deferred_waits = []
            out_acc = None
            if reps > 1:
                out_acc = small.tile([128, NH, OD], f32, tag="out_acc")
                nc.vector.memset(out_acc[:], 0.0)
            s_part = small.tile([128, NH * OD + 1], f32, tag="s_part")
            nc.vector.memset(s_part[:, NH * OD:], 0.0)
            rsum = small.tile([128, NCHUNK, N_OUT], f32, tag="rsum")

            def bcast_inner(ap, n):
                # append a 0-stride inner dim of size n
                return bass.AP(tensor=ap.tensor, offset=ap.offset,
                               ap=list(ap.ap) + [[0, n]])

            # ---- routing iterations -----------------------------------------
            for rep in range(reps):
              if rep > 0:
                # reset b/c as functions of the previous rep's s_in so that
                # reps serialize (no dead-code elim, no cross-rep overlap)
                nc.vector.tensor_scalar_mul(
                    b_rep[:],
                    bass.AP(tensor=s_in[:].tensor, offset=s_in[:].offset,
                            ap=[s_in[:].ap[0], [0, NCHUNK], [1, 16]]),
                    0.0)
                nc.vector.tensor_scalar(
                    out=c_rep[:, :, :N_OUT],
                    in0=bass.AP(tensor=s_in[:].tensor, offset=s_in[:].offset,
                                ap=[s_in[:].ap[0], [0, NCHUNK], [1, N_OUT]]),
                    scalar1=0.0, scalar2=1.0 / N_OUT,
                    op0=mybir.AluOpType.mult, op1=mybir.AluOpType.add)
              for r in range(ROUTING_ITERS + 1):
                  if r > 0:
                      # softmax over o on b_rep[:, :, :10] -> c_rep[:, :, :10]
                      bv = b_rep[:, :, :N_OUT]
                      negm = tmp.tile([128, NCHUNK], f32, tag="negm")
                      nc.vector.reduce_max(out=negm[:], in_=bv,
                                           axis=mybir.AxisListType.X, negate=True)
                      sh = tmp.tile([128, NCHUNK, N_OUT], f32, tag="shift")
                      nc.vector.tensor_add(sh[:], bv, bcast_inner(negm[:], N_OUT))
                      ex = tmp.tile([128, NCHUNK, N_OUT], f32, tag="ex")
                      nc.scalar.activation(out=ex[:], in_=sh[:],
                                           func=mybir.ActivationFunctionType.Exp)
                      ssum = tmp.tile([128, NCHUNK], f32, tag="ssum")
                      nc.vector.reduce_sum(out=ssum[:], in_=ex[:],
                                           axis=mybir.AxisListType.X)
                      rec = tmp.tile([128, NCHUNK], f32, tag="rec")
                      nc.vector.reciprocal(out=rec[:], in_=ssum[:])
                      nc.vector.tensor_mul(c_rep[:, :, :N_OUT], ex[:],
                                           bcast_inner(rec[:], N_OUT))

                  # A2[c][:, :160] = W2[c] * c_rep[:, c, o] (broadcast over d)
                  for c in range(NCHUNK):
                      nc.vector.tensor_mul(
                          A2[c][:, :OD].rearrange("p (o d) -> p o d", d=D_OUT),
                          W2[c][:].rearrange("p (o d) -> p o d", d=D_OUT),
                          bcast_inner(c_rep[:, c, :N_OUT], D_OUT))

                  # s_partial[b, od] = sum_j xT[j, b] * A2[j, od]
                  s_ps = []
                  for h in range(NH):
                      sp = ps_s.tile([128, FREE], f32, tag=f"s_ps{h}")
                      for c in range(NCHUNK):
                          nc.tensor.matmul(out=sp[:],
                                           lhsT=XT[c][:, h * 128:(h + 1) * 128],
                                           rhs=A2[c][:, :FREE],
                                           start=(c == 0), stop=(c == NCHUNK - 1))
                      s_ps.append(sp)
                  for h in range(NH):
                      nc.vector.tensor_copy(out=s_part[:, h * OD:(h + 1) * OD],
                                            in_=s_ps[h][:, :OD])

                  # AllReduce over the 8 cores
                  if crit_only:
                      for _ci in range(15):
                          with tc.tile_critical():
                              nc.vector.tensor_copy(
                                  out=s_part[0:1, 0:1], in_=s_part[0:1, 0:1])
                  if use_rdma:
                      # XOR-tree all-reduce over remote SBUF-to-SBUF DMA:
                      # 3 rounds; in round k exchange the running sum with the
                      # core at tpb XOR 2^k and add. Payload split in 4 chunks
                      # across DMA-lane slots for parallel engines. Remote-sem
                      # waits are injected after Tile scheduling (the local
                      # scheduling sim cannot model cross-core increments).
                      t_iter = ar_count[0]
                      par = t_iter % 2
                      n_par = t_iter // 2 + 1
                      NSPLIT = 4
                      CH = NH * OD // NSPLIT
                      DC = NH * OD          # dummy ordering column index
                      accs = [s_part[:], acc1[:], acc2[:], s_in_ext[:]]
                      for k in range(3):
                          delta = 1 << k
                          slots = [0, 1, 2, 3] if delta < 4 else [4, 5, 6, 7]
                          srcv = accs[k]
                          rbv = rbuf[(par, k)][:]
                          for ci, sl in enumerate(slots):
                              rd_i = [None] * 8
                              rd_i[sl] = (0, delta)
                              nc.gpsimd.remote_dma_broadcast(
                                  out_ap=rbv[:, ci * CH:(ci + 1) * CH],
                                  in_ap=srcv[:, ci * CH:(ci + 1) * CH],
                                  remote_sem=rsems[k][par],
                                  local_sem=rdma_lsem,
                                  rdests=rd_i)
                          nc.gpsimd.trigger_dma(count=None)
                          # arrival gate: copy into the dummy column; only
                          # vector-engine deps, so its wait slot is free for
                          # the injected remote-sem wait. Reading srcv[0,0]
                          # (just produced on the vector engine) keeps the
                          # gate AFTER this round's producer so the vector
                          # engine cannot block here before issuing the work
                          # the peers depend on; the add reads the dummy
                          # column -> same-engine RAW pins gate->add.
                          g1 = nc.vector.tensor_copy(out=rbv[0:1, DC:DC + 1],
                                                     in_=srcv[0:1, 0:1])
                          deferred_waits.append((g1, rsems[k][par], 8 * n_par))
                          if k == 1 and t_iter > 0:
                              # acc2 is rewritten by this add; wait for the
                              # previous iteration's round-2 sends of acc2.
                              g3 = nc.vector.tensor_copy(
                                  out=rbv[0:1, DC:DC + 1],
                                  in_=srcv[0:1, 0:1])
                              deferred_waits.append(
                                  (g3, rdma_lsem, 16 * 3 * NSPLIT * t_iter))
                          if k == 2:
                              # everything of the NEXT iteration is ordered
                              # after this add (via the s chain); wait for this
                              # iteration's round-0/1 sends so s_part and acc1
                              # can be safely rewritten then.
                              g2 = nc.vector.tensor_copy(
                                  out=rbv[0:1, DC:DC + 1],
                                  in_=srcv[0:1, 0:1])
                              deferred_waits.append(
                                  (g2, rdma_lsem,
                                   16 * (3 * NSPLIT * t_iter + 2 * NSPLIT)))
                          nc.vector.tensor_add(accs[k + 1], srcv, rbv)
                      ar_count[0] += 1
                  else:
                    ar_in = dram.tile([128, NH * OD], f32, tag="ar_in")
                    ar_out = dram.tile([128, NH * OD], f32, tag="ar_out")
                    nc.sync.dma_start(out=ar_in[:], in_=s_part[:, :NH * OD])
                    if no_ar:
                        nc.sync.dma_start(out=ar_out[:], in_=ar_in[:])
                    elif ar_tiny:
                        # timing probe: collective floor with a 2KB payload
                        tin = dram.tile([128, 4], f32, tag="tin")
                        tout = dram.tile([128, 4], f32, tag="tout")
                        nc.sync.dma_start(out=tin[:], in_=s_part[:, :4])
                        nc.gpsimd.collective_compute(
                            "AllReduce", mybir.AluOpType.add,
                            replica_groups=[list(range(N_CORES))],
                            ins=[tin.opt()], outs=[tout.opt()])
                        nc.sync.dma_start(out=ar_out[:, :4], in_=tout[:])
                        nc.sync.dma_start(out=ar_out[:, 4:], in_=ar_in[:, 4:])
                    elif use_ag:
                        ag_out = dram.tile([128 * N_CORES, NH * OD], f32,
                                           tag="ag_out")
                        nc.gpsimd.collective_compute(
                            "AllGather", mybir.AluOpType.bypass,
                            replica_groups=[list(range(N_CORES))],
                            ins=[ar_in.opt()], outs=[ag_out.opt()])
                        gsum = tmp.tile([128, N_CORES, NH * OD], f32, tag="gsum")
                        nc.sync.dma_start(
                            out=gsum[:],
                            in_=ag_out[:].rearrange("(n p) f -> p n f", p=128))
                        a01 = tmp.tile([128, NH * OD], f32, tag="ag_a01")
                        nc.vector.tensor_add(a01[:], gsum[:, 0, :], gsum[:, 1, :])
                        a23 = tmp.tile([128, NH * OD], f32, tag="ag_a23")
                        nc.vector.tensor_add(a23[:], gsum[:, 2, :], gsum[:, 3, :])
                        a45 = tmp.tile([128, NH * OD], f32, tag="ag_a45")
                        nc.vector.tensor_add(a45[:], gsum[:, 4, :], gsum[:, 5, :])
                        a67 = tmp.tile([128, NH * OD], f32, tag="ag_a67")
                        nc.vector.tensor_add(a67[:], gsum[:, 6, :], gsum[:, 7, :])
                        q01 = tmp.tile([128, NH * OD], f32, tag="ag_q01")
                        nc.vector.tensor_add(q01[:], a01[:], a23[:])
                        q45 = tmp.tile([128, NH * OD], f32, tag="ag_q45")
                        nc.vector.tensor_add(q45[:], a45[:], a67[:])
                        nc.vector.tensor_add(s_in[:].rearrange("p h od -> p (h od)"), q01[:], q45[:])
                    else:
                        nc.gpsimd.collective_compute(
                            "AllReduce", mybir.AluOpType.add,
                            replica_groups=[list(range(N_CORES))],
                            ins=[ar_in.opt()], outs=[ar_out.opt()])
                    if not use_ag:
                        nc.sync.dma_start(
                            out=s_in[:].rearrange("p h od -> p (h od)"),
                            in_=ar_out[:])

                  # squash: s = s * l2/(1+l2^2) per (b, o) over d
                  sv = s_in[:].rearrange("p h (o d) -> p h o d", d=D_OUT)
                  sq = tmp.tile([128, NH, N_OUT, D_OUT], f32, tag="sq")
                  nc.scalar.square(sq[:], sv)
                  q = tmp.tile([128, NH, N_OUT], f32, tag="q")
                  nc.vector.reduce_sum(out=q[:], in_=sq[:],
                                       axis=mybir.AxisListType.X)
                  l2 = tmp.tile([128, NH, N_OUT], f32, tag="l2")
                  nc.scalar.sqrt(l2[:], q[:])
                  qp1 = tmp.tile([128, NH, N_OUT], f32, tag="qp1")
                  nc.vector.tensor_scalar_add(qp1[:], q[:], 1.0)
                  rec2 = tmp.tile([128, NH, N_OUT], f32, tag="rec2")
                  nc.vector.reciprocal(out=rec2[:], in_=qp1[:])
                  g = tmp.tile([128, NH, N_OUT], f32, tag="g")
                  nc.vector.tensor_mul(g[:], l2[:], rec2[:])

                  if r == ROUTING_ITERS:
                      nc.vector.tensor_mul(sv, sv, bcast_inner(g[:], D_OUT))
                      if reps > 1:
                          nc.vector.tensor_add(out_acc[:], out_acc[:], s_in[:])
                      if rep == reps - 1:
                          src_t = out_acc if reps > 1 else s_in
                          for h in range(NH):
                              nc.sync.dma_start(
                                  out=out_d[h * 128:(h + 1) * 128, :, :],
                                  in_=src_t[:, h, :])
                      break

                  svo = s_full[:, :, :OD].rearrange("p h (o d) -> p h o d", d=D_OUT)
                  nc.vector.tensor_mul(svo, sv, bcast_inner(g[:], D_OUT))

                  # T[j, od] = sum_b x[b, j] * s[b, od]; b_upd via W2*T reductions
                  for c in range(NCHUNK):
                      tp = ps_t.tile([128, FREE], f32, tag="t_ps")
                      for h in range(NH):
                          nc.tensor.matmul(out=tp[:],
                                           lhsT=X[h][:, c * 128:(c + 1) * 128],
                                           rhs=s_full[:, h, :FREE],
                                           start=(h == 0), stop=(h == NH - 1))
                      pr = tmp.tile([128, OD], f32, tag="pr")
                      nc.vector.tensor_mul(pr[:], W2[c][:], tp[:, :OD])
                      nc.vector.reduce_sum(
                          out=rsum[:, c, :],
                          in_=pr[:].rearrange("p (o d) -> p o d", d=D_OUT),
                          axis=mybir.AxisListType.X)

                  # group-sum over k (8 partitions) + broadcast back: BlockOnes mm
                  bu = ps_b.tile([128, NCHUNK * N_OUT], f32, tag="bu")
                  nc.tensor.matmul(out=bu[:], lhsT=bones[:], rhs=rsum[:],
                                   start=True, stop=True)
                  nc.vector.tensor_add(
                      b_rep[:, :, :N_OUT], b_rep[:, :, :N_OUT],
                      bu[:].rearrange("p (c o) -> p c o", o=N_OUT))

    if use_rdma:
        for inst, sem, val in deferred_waits:
            inst.wait_op(sem, val, "sem-ge", check=False)
    nc.finalize()
    return nc


def _get_nc(mm_mode, reps=1):
    key = (mm_mode, reps)
    if key not in _cache:
        _cache[key] = _build(mm_mode, reps)
    return _cache[key]


_BONES = np.kron(np.eye(16, dtype=np.float32),
                 np.ones((8, 8), dtype=np.float32))


def _shard(x, W):
    x = np.ascontiguousarray(x, dtype=np.float32)
    W = np.ascontiguousarray(W, dtype=np.float32)
    in_maps = []
    for r in range(N_CORES):
        xs = x[:, r * I_LOC:(r + 1) * I_LOC, :]          # [B, 144, 8]
        x_r = np.ascontiguousarray(xs.reshape(B, J))
        xt_r = np.ascontiguousarray(x_r.T)
        Ws = W[r * I_LOC:(r + 1) * I_LOC]                # [144, 10, 16, 8]
        w2_r = np.ascontiguousarray(
            Ws.transpose(0, 3, 1, 2).reshape(J, OD))     # [(i,k), (o,d)]
        in_maps.append({"x": x_r, "xt": xt_r, "w2": w2_r, "bones": _BONES})
    return in_maps


def run(x, W, trace=False, mm_mode=MM_MODE):
    from concourse.bass_utils import run_bass_kernel_spmd
    nc = _get_nc(mm_mode)
    in_maps = _shard(x, W)
    res = run_bass_kernel_spmd(nc, in_maps, core_ids=list(range(N_CORES)),
                               trace=trace)
    return res.results[0]["out"].astype(np.float32), res


def kernel(x, W):
    out, _ = run(x, W)
    return out

